# revision 31
# baseline (speedup 1.0000x reference)
"""Trainium2 Bass kernel for sparse attention with relation bias.

Computes, for inputs (B=4, N=512, C=128, H=8, HS=16):
  qkv = joint @ W_qkv^T -> q,k,v
  attn = softmax((q k^T + rel @ W_r^T) * conn * HS^-0.5)
  out  = (attn @ v) @ W_proj^T + b_proj

Sharding: 8 cores, core i handles batch b=i//2 and n-row half i%2 (256 rows).
No collectives — each core computes its own output rows; host gathers.

Layout notes:
- All engine SBUF APs must start at a 32-aligned partition, so heads (HS=16)
  are padded to 32 partitions and split into two half-tensors (heads 0-3 "A",
  heads 4-7 "B") for q/k/v/x. Weights are host-padded to match.
- Attention PSUM layout per subgroup of 4 n-rows: partition = jj*32 + h.
- rel is cast f32->bf16 during the SWDGE DMA; all matmuls run in bf16 with
  f32 PSUM accumulation.
"""

import sys

sys.path.insert(0, "/opt/trn_rl_repo")

import numpy as np

import concourse.bass as bass
import concourse.tile as tile
from concourse import bacc, mybir
from concourse.masks import make_identity
from contextlib import ExitStack

F32 = mybir.dt.float32
F32R = mybir.dt.float32r
BF16 = mybir.dt.bfloat16

# Problem constants (hardcoded per spec)
B, N, C, H = 4, 512, 128, 8
HS = C // H  # 16
SCALE = float(HS) ** -0.5
NCORES = 8
P = 128  # partitions
MC = N // P  # m-chunks per row = 4
HH = H // 2  # heads per half = 4

# wqkvT_pad column sections: qA qB kA kB vA vB, each 128 wide
QA, QB, KA, KB, VA, VB = (i * P for i in range(6))

# Use the DMA xbar (dma_start_transpose) instead of PE transposes
import os as _os

XBAR = _os.environ.get("BASS_XBAR_REL", "0") == "1"


def build_graph(NH, G=16):
    """Build the SPMD single-core graph. NH = n-rows per core."""
    NG = NH // G  # load groups
    TPG = G * MC  # [128,128] rel tiles per load group (64)

    nc = bacc.Bacc("TRN2", target_bir_lowering=False, debug=False)
    rel_d = nc.declare_dram_parameter("rel", [NH * N, C], F32, isOutput=False)
    conn_d = nc.declare_dram_parameter("conn", [NH, N], F32, isOutput=False)
    jT_d = nc.declare_dram_parameter("jointT", [C, N], F32, isOutput=False)
    jTq_d = nc.declare_dram_parameter("jointTq", [C, NH], F32, isOutput=False)
    wqkvT_d = nc.declare_dram_parameter("wqkvT", [C, 6 * P], F32, isOutput=False)
    wrT_d = nc.declare_dram_parameter("wrT", [C, H], F32, isOutput=False)
    wpT_d = nc.declare_dram_parameter("wprojT", [C, 2 * C], F32, isOutput=False)
    bp_d = nc.declare_dram_parameter("bproj", [1, C], F32, isOutput=False)
    out_d = nc.declare_dram_parameter("out", [NH, C], F32, isOutput=True)

    with tile.TileContext(nc) as tc, ExitStack() as ctx:
        singles = ctx.enter_context(tc.tile_pool(name="singles", bufs=1))
        relpool = ctx.enter_context(tc.tile_pool(name="relpool", bufs=3))
        relTp = ctx.enter_context(tc.tile_pool(name="relTp", bufs=3))
        connp = ctx.enter_context(tc.tile_pool(name="connp", bufs=3))
        logitp = ctx.enter_context(tc.tile_pool(name="logitp", bufs=3))
        attnwp = ctx.enter_context(tc.tile_pool(name="attnwp", bufs=6))
        aTp = ctx.enter_context(tc.tile_pool(name="aTp", bufs=2))
        smallp = ctx.enter_context(tc.tile_pool(name="smallp", bufs=10))
        outp = ctx.enter_context(tc.tile_pool(name="outp", bufs=3))

        ps_attn = ctx.enter_context(
            tc.tile_pool(name="ps_attn", bufs=4 if XBAR else 3, space="PSUM")
        )
        if not XBAR:
            # relT and aT transposes share one 2-slot pool (same shape/phase rotation)
            ps_relT = ctx.enter_context(
                tc.tile_pool(name="ps_relT", bufs=2, space="PSUM")
            )
            ps_aT = ps_relT
        ps_x = ctx.enter_context(tc.tile_pool(name="ps_x", bufs=2, space="PSUM"))
        ps_o = ctx.enter_context(tc.tile_pool(name="ps_o", bufs=1, space="PSUM"))

        # ---- constants / weights (DMA-cast f32 -> bf16 via SWDGE) ----
        ident = singles.tile([P, P], BF16)
        make_identity(nc, ident)
        wqkvT = singles.tile([P, 6 * P], BF16)
        nc.gpsimd.dma_start(out=wqkvT, in_=wqkvT_d[:, :])
        wrT = singles.tile([P, H], BF16)
        nc.gpsimd.dma_start(out=wrT, in_=wrT_d[:, :])
        wpT = singles.tile([P, 2 * C], BF16)
        nc.gpsimd.dma_start(out=wpT, in_=wpT_d[:, :])
        bp = singles.tile([1, C], BF16)
        nc.gpsimd.dma_start(out=bp, in_=bp_d[:, :])
        ones = singles.tile([1, G], BF16)
        nc.vector.memset(ones, 1.0)
        jT = singles.tile([P, N], BF16)
        nc.gpsimd.dma_start(out=jT, in_=jT_d[:, :])
        jTq = singles.tile([P, NH], BF16)
        nc.gpsimd.dma_start(out=jTq, in_=jTq_d[:, :])
        QexpAs, QexpBs = [], []
        for _i in range(2):
            qa = singles.tile([P, 4 * P], BF16, tag=f"qexpa{_i}")
            nc.vector.memset(qa, 0.0)
            QexpAs.append(qa)
            qb = singles.tile([P, 4 * P], BF16, tag=f"qexpb{_i}")
            nc.vector.memset(qb, 0.0)
            QexpBs.append(qb)
        xTA = singles.tile([P, G], BF16)
        nc.vector.memset(xTA, 0.0)
        xTB = singles.tile([P, G], BF16)
        nc.vector.memset(xTB, 0.0)

        kTA = singles.tile([P, N], BF16)
        kTB = singles.tile([P, N], BF16)
        vnatA = singles.tile([P, MC, P], BF16)
        vnatB = singles.tile([P, MC, P], BF16)
        qTA = singles.tile([P, NH], BF16)
        qTB = singles.tile([P, NH], BF16)

        # ---- prep: qkv projections (padded head layout, halves A/B) ----
        for dst, col in ((kTA, KA), (kTB, KB)):
            pk = ps_attn.tile([P, N], F32, tag="attn")
            nc.tensor.matmul(
                pk, lhsT=wqkvT[:, col : col + P], rhs=jT, start=True, stop=True
            )
            nc.vector.tensor_copy(dst, pk)
        for dst, col in ((vnatA, VA), (vnatB, VB)):
            for t in range(MC):
                pv = ps_attn.tile([P, N], F32, tag="attn")
                nc.tensor.matmul(
                    pv[:, :P],
                    lhsT=jT[:, t * P : (t + 1) * P],
                    rhs=wqkvT[:, col : col + P],
                    start=True,
                    stop=True,
                )
                nc.vector.tensor_copy(dst[:, t, :], pv[:, :P])
        for dst, col in ((qTA, QA), (qTB, QB)):
            pq = ps_attn.tile([P, N], F32, tag="attn")
            nc.tensor.matmul(
                pq[:, :NH], lhsT=wqkvT[:, col : col + P], rhs=jTq, start=True, stop=True
            )
            nc.vector.tensor_copy(dst, pq[:, :NH])

        rel_view = rel_d[:, :].rearrange("(t p) c -> p t c", p=P)

        # alternate PSUM->SBUF copies between DVE and ACT to balance load.
        # On DVE, copy bf16 data as uint32 (half the element count — bf16 PSUM
        # sources get no perf-mode acceleration, u32 halves the stream).
        def copy_alt(i, out, in_):
            if i % 2 == 0:
                if out.dtype == BF16 and in_.dtype == BF16:
                    nc.vector.tensor_copy(
                        out.bitcast(mybir.dt.uint32), in_.bitcast(mybir.dt.uint32)
                    )
                else:
                    nc.vector.tensor_copy(out, in_)
            else:
                nc.scalar.copy(out, in_)

        for g in range(NG):
            rel_sb = relpool.tile([P, TPG, P], BF16)
            nc.gpsimd.dma_start(out=rel_sb, in_=rel_view[:, g * TPG : (g + 1) * TPG, :])

            # Qexp strips: QexpX[hp*32+dh, s*128+jj*32+h] = qTX[hp*32+dh, n(s,jj)]
            QexpA, QexpB = QexpAs[g % 2], QexpBs[g % 2]
            qvA = QexpA.rearrange("p (s jj r) -> p s jj r", s=4, r=32)
            qvB = QexpB.rearrange("p (s jj r) -> p s jj r", s=4, r=32)
            for hp in range(HH):
                for qv, qTx, h in ((qvA, qTA, hp), (qvB, qTB, hp + HH)):
                    nc.gpsimd.tensor_copy(
                        out=qv[hp * 32 : hp * 32 + HS, :, :, h],
                        in_=qTx[
                            hp * 32 : hp * 32 + HS, g * G : (g + 1) * G
                        ].rearrange("p (s jj) -> p s jj", jj=4),
                    )

            attn_ws = []
            for s in range(4):
                n0 = g * G + s * 4
                conn_e = connp.tile([P, N], BF16)
                conn_src = conn_d[n0 : n0 + 4, :]
                conn_bcast = bass.AP(
                    tensor=conn_src.tensor,
                    offset=conn_src.offset,
                    ap=[conn_src.ap[0], [0, 32], conn_src.ap[1]],
                )
                nc.gpsimd.dma_start(out=conn_e[:, :], in_=conn_bcast)

                Pattn = ps_attn.tile([P, N], F32, tag="attn")
                nc.tensor.matmul(
                    Pattn,
                    lhsT=QexpA[:, s * P : (s + 1) * P],
                    rhs=kTA,
                    start=True,
                    stop=False,
                )
                nc.tensor.matmul(
                    Pattn,
                    lhsT=QexpB[:, s * P : (s + 1) * P],
                    rhs=kTB,
                    start=False,
                    stop=True,
                )
                for jj in range(4):
                    j = s * 4 + jj
                    relT = relTp.tile([P, N], BF16)
                    PT = ps_relT.tile([P, N], BF16, tag="tp")
                    for mc in range(MC):
                        nc.tensor.transpose(
                            PT[:, mc * P : (mc + 1) * P],
                            rel_sb[:, j * MC + mc, :],
                            ident,
                        )
                    copy_alt(j, relT, PT)
                    nc.tensor.matmul(
                        Pattn[jj * 32 : jj * 32 + H, :],
                        lhsT=wrT,
                        rhs=relT,
                        start=False,
                        stop=True,
                        tile_position=(0, jj * 32),
                        skip_group_check=True,
                    )

                logits = logitp.tile([P, N], BF16)
                nc.vector.tensor_mul(logits, Pattn, conn_e)
                attn_w = attnwp.tile([P, N], BF16)
                sums = smallp.tile([P, 1], F32)
                nc.scalar.activation(
                    out=attn_w,
                    in_=logits,
                    func=mybir.ActivationFunctionType.Exp,
                    scale=SCALE,
                    accum_out=sums,
                )
                recip = smallp.tile([P, 1], F32)
                nc.vector.reciprocal(recip, sums)
                nc.vector.tensor_scalar_mul(attn_w, attn_w, recip)
                attn_ws.append(attn_w)

            # attn @ v, accumulated over m-chunks; PXx[hp*32+dh, s*128+jj*32+h]
            PX = ps_x.tile([P, 2 * P], F32, tag="px")
            PXA = PX[:, 0:P]
            PXB = PX[:, P : 2 * P]
            for c in range(MC):
                aT = aTp.tile([P, N], BF16)
                PA = ps_aT.tile([P, N], BF16, tag="tp")
                for s in range(4):
                    nc.tensor.transpose(
                        PA[:, s * P : (s + 1) * P],
                        attn_ws[s][:, c * P : (c + 1) * P],
                        ident,
                    )
                copy_alt(c, aT, PA)
                # rhs skips the 24 pad columns per 32-block: N=128 instead of 512
                aT_tight = aT.rearrange("p (s jj r) -> p s jj r", s=4, r=32)[
                    :, :, :, 0:H
                ]
                nc.tensor.matmul(
                    PXA,
                    lhsT=vnatA[:, c, :],
                    rhs=aT_tight,
                    start=(c == 0),
                    stop=(c == MC - 1),
                )
                nc.tensor.matmul(
                    PXB,
                    lhsT=vnatB[:, c, :],
                    rhs=aT_tight,
                    start=False,
                    stop=(c == MC - 1),
                    skip_group_check=True,
                )

            # extract xTx[hp*32+dh, (s,jj)] = PXx[hp*32+dh, (4s+jj)*32 + h]
            pxvA = PXA.rearrange("p (q r) -> p q r", r=H)
            pxvB = PXB.rearrange("p (q r) -> p q r", r=H)
            for hp in range(HH):
                sl = slice(hp * 32, hp * 32 + HS)
                copy_alt(hp, xTA[sl, :], pxvA[sl, :, hp])
                copy_alt(hp + 1, xTB[sl, :], pxvB[sl, :, hp + HH])

            PO = ps_o.tile([G, C], F32)
            nc.tensor.matmul(PO, lhsT=xTA, rhs=wpT[:, :C], start=True, stop=False)
            nc.tensor.matmul(PO, lhsT=xTB, rhs=wpT[:, C:], start=False, stop=False)
            nc.tensor.matmul(PO, lhsT=ones, rhs=bp, start=False, stop=True)
            out_sb = outp.tile([G, C], F32)
            nc.vector.tensor_copy(out_sb, PO)
            nc.sync.dma_start(out=out_d[g * G : (g + 1) * G, :], in_=out_sb)

    return nc


_GRAPH_CACHE = {}


def _get_graph(NH):
    if NH not in _GRAPH_CACHE:
        nc = build_graph(NH)
        nc.finalize()
        _GRAPH_CACHE[NH] = nc
    return _GRAPH_CACHE[NH]


def _pad_heads(W):
    """[H*HS, C] -> two padded [4*32, C] halves (heads 0-3, 4-7), zeros in pad rows."""
    Wr = W.reshape(H, HS, -1)
    out = []
    for half in range(2):
        pad = np.zeros((HH, 32, W.shape[-1]), dtype=W.dtype)
        pad[:, :HS] = Wr[half * HH : (half + 1) * HH]
        out.append(pad.reshape(HH * 32, -1))
    return out


def make_in_maps(joint_feature, relation_feature, conn_feature, W_qkv, W_r, W_proj, b_proj):
    """Shard full inputs into 8 per-core input maps."""
    NH = N // 2
    Wq, Wk, Wv = W_qkv[:C], W_qkv[C : 2 * C], W_qkv[2 * C :]
    qA, qB = _pad_heads(Wq)
    kA, kB = _pad_heads(Wk)
    vA, vB = _pad_heads(Wv)
    # wqkvT: [C, 6*128] — sections qA qB kA kB vA vB (transposed)
    wqkvT = np.ascontiguousarray(np.concatenate([qA, qB, kA, kB, vA, vB], axis=0).T)
    # wprojT: [C(padded in-space per half), 2*C]
    WpT = W_proj.T  # [c_in, c_out]
    pA, pB = _pad_heads(WpT)  # pads c_in (= head space of x)
    wpT = np.ascontiguousarray(np.concatenate([pA, pB], axis=1))
    wrT = np.ascontiguousarray(W_r.T)
    bp = np.ascontiguousarray(b_proj[None, :])
    in_maps = []
    for core in range(NCORES):
        b = core // 2
        half = core % 2
        n0 = half * NH
        jT = np.ascontiguousarray(joint_feature[b].T)
        jTq = np.ascontiguousarray(joint_feature[b, n0 : n0 + NH].T)
        rel = np.ascontiguousarray(relation_feature[b, n0 : n0 + NH].reshape(NH * N, C))
        conn = np.ascontiguousarray(conn_feature[b, n0 : n0 + NH])
        in_maps.append(
            {
                "rel": rel,
                "conn": conn,
                "jointT": jT,
                "jointTq": jTq,
                "wqkvT": wqkvT,
                "wrT": wrT,
                "wprojT": wpT,
                "bproj": bp,
            }
        )
    return in_maps


def kernel(joint_feature, relation_feature, conn_feature, W_qkv, W_r, W_proj, b_proj):
    from concourse.bass_utils import run_bass_kernel_spmd

    NH = N // 2
    nc = _get_graph(NH)
    in_maps = make_in_maps(
        joint_feature, relation_feature, conn_feature, W_qkv, W_r, W_proj, b_proj
    )
    res = run_bass_kernel_spmd(nc, in_maps, core_ids=list(range(NCORES)))
    out = np.zeros((B, N, C), dtype=np.float32)
    for core in range(NCORES):
        b = core // 2
        half = core % 2
        n0 = half * NH
        out[b, n0 : n0 + NH] = res.results[core]["out"]
    return out
